# revision 1
# baseline (speedup 1.0000x reference)
"""2-layer GraphConv (PyG-style) on 8 TRN2 NeuronCores via Bass/Tile.

Strategy (dst-sharded, SPMD, one NEFF):
  - Nodes sharded across 8 cores (2500 rows each). Weights replicated.
  - Per layer L: y = x @ W_rel computed per-shard, AllGather -> y_full [N,128] in DRAM.
    agg[d] = sum_{(s,d) in E} y[s]  done per-core over its own dst rows:
      edges grouped by dst into 64-dst "halves"; each half's edges are packed
      into chunks of 128 slots (padded); per chunk a one-hot S [128e x 64d]
      (built on DVE via is_equal against an iota) maps edge-slots to dst
      partitions, and PE matmul psum[wp:wp+64, :] += S.T @ Xg accumulates.
      Xg tiles come from dma_gather (SWDGE row gather from y_full).
    Root term (x @ W_root) and bias are folded into the same PSUM accumulation
    group as matmuls (bias via a K=1 ones-row matmul).
  - Chunk counts per half are maxed across cores so one instruction stream
    serves all cores (padding slots gather row 0 and carry D=-1 -> S row = 0).
"""

import json as _json
import os as _os
import shlex as _shlex


def _apply_cc_workaround():
    """Skip neuronxcc's optional DataLocalityOpt pass: it hits an internal
    assert (NCC_IDLO901) trying to prefetch-localize the 10MB shared gather
    source. Must run before the jax/axon backend captures compile flags."""
    skip = "--skip-pass=InsertConflictResolutionOps|DataLocalityOpt"

    def fix(flags):
        # --skip-pass is a tensorizer option: splice it into the existing
        # --tensorizer-options= string (top-level it is rejected, NCC_EARG002)
        out = []
        for f in flags:
            if f == skip:
                continue  # drop broken standalone form from earlier attempts
            if f.startswith("--tensorizer-options=") and skip not in f:
                f = f.rstrip() + " " + skip + " "
            out.append(f)
        return out

    pc_path = _os.environ.get("TRN_TERMINAL_PRECOMPUTED_JSON")
    flags = None
    if pc_path and _os.path.exists(pc_path):
        pc = _json.load(open(pc_path))
        pc["cc_flags"] = fix(pc.get("cc_flags", []))
        _json.dump(pc, open(pc_path, "w"))
        flags = list(pc["cc_flags"])
    try:
        from concourse.compiler_utils import (get_compiler_flags,
                                              set_compiler_flags)
        fl = fix(get_compiler_flags())
        set_compiler_flags(fl)
        if fl:
            _os.environ["NEURON_CC_FLAGS"] = _shlex.join(fl)
    except Exception:
        if flags is not None:
            _os.environ["NEURON_CC_FLAGS"] = _shlex.join(flags)


_apply_cc_workaround()

import numpy as np

import concourse.bacc as bacc
import concourse.bass as bass
import concourse.mybir as mybir
import concourse.tile as tile
from concourse.bass import AP
from concourse.bass_utils import run_bass_kernel_spmd
from concourse.masks import make_identity

F32 = mybir.dt.float32
I16 = mybir.dt.int16
P = 128          # partitions / chunk slot count
W = 64           # dst window width (half-tile)
D = 128          # feature dim


def cdiv(a, b):
    return (a + b - 1) // b


# ---------------------------------------------------------------------------
# Host-side preprocessing: edge grouping -> per-core gather indices + D values
# ---------------------------------------------------------------------------

def preprocess(edge_index, n_nodes, n_cores):
    """Returns (meta, per_core) where
    meta = dict(npc, n_tiles, tiles=[(rows, [wp ...])], n_chunks_total, n_slots)
    per_core[c] = dict(gidx=[128, n_slots//16] int16, dval=[128, n_chunks] f32)
    """
    src = np.asarray(edge_index[0]).astype(np.int64)
    dst = np.asarray(edge_index[1]).astype(np.int64)
    npc = n_nodes // n_cores
    n_halves = cdiv(npc, W)
    n_tiles = cdiv(npc, P)

    owner = dst // npc
    dloc = dst - owner * npc
    half = dloc // W

    # per (core, half) edge srcs
    key = owner * n_halves + half
    order = np.argsort(key, kind="stable")
    key_s = key[order]
    src_s = src[order]
    dloc_s = dloc[order]
    bounds = np.searchsorted(key_s, np.arange(n_cores * n_halves + 1))

    counts = (bounds[1:] - bounds[:-1]).reshape(n_cores, n_halves)
    chunks_per_half = np.maximum(cdiv(counts.max(axis=0), P), 0)  # [n_halves]

    n_chunks_total = int(chunks_per_half.sum())
    n_slots = n_chunks_total * P

    per_core = []
    for c in range(n_cores):
        gidx = np.zeros(n_slots, dtype=np.int16)
        dval = np.full(n_slots, -1.0, dtype=np.float32)
        s0 = 0
        for h in range(n_halves):
            nch = int(chunks_per_half[h])
            if nch == 0:
                continue
            b0, b1 = bounds[c * n_halves + h], bounds[c * n_halves + h + 1]
            cnt = b1 - b0
            gidx[s0:s0 + cnt] = src_s[b0:b1]
            dval[s0:s0 + cnt] = (dloc_s[b0:b1] - h * W).astype(np.float32)
            s0 += nch * P
        assert s0 == n_slots
        # dma_gather index layout: idx i at [i % 16, i // 16], replicated x8
        g16 = gidx.reshape(-1, 16).T  # [16, n_slots//16]
        gidx_l = np.tile(g16, (8, 1)).astype(np.int16)
        # D layout: slot k*P + p at [p, k]
        dv_l = dval.reshape(n_chunks_total, P).T.astype(np.float32)
        per_core.append({"gidx": np.ascontiguousarray(gidx_l),
                         "dval": np.ascontiguousarray(dv_l)})

    # pad total chunk count to a multiple of 8 so every dma_gather call is
    # exactly 1024 indices (ucode limit ~1024 idxs/call); dummies go on the
    # last tile with D=-1 (S row zero) and gather row 0
    GPC = 8  # chunks per gather piece
    pad_chunks = (-n_chunks_total) % GPC
    n_chunks_total += pad_chunks
    n_slots = n_chunks_total * P
    for pc in per_core:
        gi = np.zeros((P, n_slots // 16), np.int16)
        gi[:, :pc["gidx"].shape[1]] = pc["gidx"]
        dv = np.full((P, n_chunks_total), -1.0, np.float32)
        dv[:, :pc["dval"].shape[1]] = pc["dval"]
        pc["gidx"], pc["dval"] = np.ascontiguousarray(gi), np.ascontiguousarray(dv)

    # tile metadata: tile t = halves 2t, 2t+1 ; w_parts list per tile
    tiles = []
    k = 0
    slot_off = 0
    for t in range(n_tiles):
        rows = min(P, npc - t * P)
        wps = []
        for hh in (2 * t, 2 * t + 1):
            if hh >= n_halves:
                continue
            for _ in range(int(chunks_per_half[hh])):
                wps.append((hh % 2) * W)
        if t == n_tiles - 1:
            wps = wps + [0] * pad_chunks
        tiles.append({"rows": rows, "wps": wps, "k0": k, "slot0": slot_off})
        k += len(wps)
        slot_off += len(wps) * P
    assert k == n_chunks_total

    meta = {"npc": npc, "npc_pad": n_tiles * P, "n_tiles": n_tiles,
            "tiles": tiles,
            "n_chunks_total": n_chunks_total, "n_slots": n_slots,
            "n_nodes": n_nodes, "n_cores": n_cores,
            "max_chunks_per_tile": max((len(t["wps"]) for t in tiles), default=1)}
    return meta, per_core


# ---------------------------------------------------------------------------
# Kernel builder
# ---------------------------------------------------------------------------

def _bcast3(ap2d: AP, shape3, steps3):
    """Build a 3-D read AP from a 2-D AP by inserting explicit [step, count]
    pairs for the two free dims (partition dim copied from the source)."""
    (c1, n1), (c2, n2) = steps3
    new = [ap2d.ap[0], [c1, n1], [c2, n2]]
    del shape3
    return AP(ap2d.tensor, ap2d.offset, new)


def build_kernel(meta, xg_bufs=8, s_bufs=2, psum_agg_bufs=3):
    npc = meta["npc"]
    npc_pad = meta["npc_pad"]
    n_tiles = meta["n_tiles"]
    n_nodes = meta["n_nodes"]
    n_cores = meta["n_cores"]
    n_slots = meta["n_slots"]
    nch = meta["n_chunks_total"]
    maxc = meta["max_chunks_per_tile"]

    nc = bacc.Bacc("TRN2", target_bir_lowering=False, debug=False,
                   num_devices=n_cores)

    # --- I/O ---
    xT = nc.dram_tensor("xT", [P, npc_pad], F32, kind="ExternalInput")
    w1r = nc.dram_tensor("w1r", [D, D], F32, kind="ExternalInput")
    w1o = nc.dram_tensor("w1o", [D, D], F32, kind="ExternalInput")
    w2r = nc.dram_tensor("w2r", [D, D], F32, kind="ExternalInput")
    w2o = nc.dram_tensor("w2o", [D, D], F32, kind="ExternalInput")
    b1 = nc.dram_tensor("b1", [1, D], F32, kind="ExternalInput")
    b2 = nc.dram_tensor("b2", [1, D], F32, kind="ExternalInput")
    gidx = nc.dram_tensor("gidx", [P, n_slots // 16], I16, kind="ExternalInput")
    dvals = nc.dram_tensor("dvals", [P, nch], F32, kind="ExternalInput")
    out = nc.dram_tensor("out", [npc, D], F32, kind="ExternalOutput")

    rg = [list(range(n_cores))]

    with tile.TileContext(nc) as tc:
        with (
            tc.tile_pool(name="const", bufs=1) as constp,
            tc.tile_pool(name="xg", bufs=xg_bufs) as xgp,
            tc.tile_pool(name="sp", bufs=s_bufs) as sp,
            tc.tile_pool(name="psA", bufs=psum_agg_bufs, space="PSUM") as psA,
            tc.tile_pool(name="psB", bufs=2, space="PSUM") as psB,
            tc.tile_pool(name="stage", bufs=4) as stagep,
            tc.tile_pool(name="dram", bufs=1, space="DRAM") as dram,
        ):
            # --- constants / persistent SBUF ---
            w1r_sb = constp.tile([D, D], F32)
            nc.sync.dma_start(w1r_sb[:], w1r[:])
            w1o_sb = constp.tile([D, D], F32)
            nc.sync.dma_start(w1o_sb[:], w1o[:])
            w2r_sb = constp.tile([D, D], F32)
            nc.sync.dma_start(w2r_sb[:], w2r[:])
            w2o_sb = constp.tile([D, D], F32)
            nc.sync.dma_start(w2o_sb[:], w2o[:])
            b1_sb = constp.tile([1, D], F32)
            nc.sync.dma_start(b1_sb[:], b1[:])
            b2_sb = constp.tile([1, D], F32)
            nc.sync.dma_start(b2_sb[:], b2[:])
            ones_sb = constp.tile([1, D], F32)
            nc.gpsimd.memset(ones_sb[:], 1.0)
            ident_sb = constp.tile([P, P], F32)
            make_identity(nc, ident_sb[:])
            xT_sb = constp.tile([P, npc_pad], F32)
            nc.sync.dma_start(xT_sb[:], xT[:])
            hT_sb = constp.tile([P, npc_pad], F32)
            gidx_sb = constp.tile([P, n_slots // 16], I16)
            nc.sync.dma_start(gidx_sb[:], gidx[:])
            dv_sb = constp.tile([P, nch], F32)
            nc.sync.dma_start(dv_sb[:], dvals[:])
            # iota over window offsets, as fp32: iota_w[p, w] = w
            iota_i = constp.tile([P, W], mybir.dt.int32)
            nc.gpsimd.iota(iota_i[:], pattern=[[1, W]], base=0,
                           channel_multiplier=0)
            iota_f = constp.tile([P, W], F32)
            nc.vector.tensor_copy(iota_f[:], iota_i[:])

            # --- DRAM scratch ---
            y1b = dram.tile([npc, D], F32)
            y2b = dram.tile([npc, D], F32)
            y1f = dram.tile([n_nodes, D], F32, addr_space="Shared")
            y2f = dram.tile([n_nodes, D], F32, addr_space="Shared")

            def tslice(t, rows):
                return slice(t * P, t * P + rows)

            # ---- stage A: y1 = x @ W1_rel per shard ----
            for t in range(n_tiles):
                rows = meta["tiles"][t]["rows"]
                ps = psB.tile([P, D], F32, tag="psB")
                nc.tensor.matmul(ps[:rows, :], lhsT=xT_sb[:, tslice(t, rows)],
                                 rhs=w1r_sb[:], start=True, stop=True)
                y_sb = stagep.tile([P, D], F32, tag="ystage")
                nc.scalar.activation(y_sb[:rows, :], ps[:rows, :],
                                     mybir.ActivationFunctionType.Copy)
                nc.sync.dma_start(y1b[tslice(t, rows), :], y_sb[:rows, :])

            nc.gpsimd.collective_compute(
                "AllGather", mybir.AluOpType.bypass, replica_groups=rg,
                ins=[y1b.opt()], outs=[y1f.opt()])

            # ---- helper: one aggregation pass ----
            GPC = 8  # chunks per 1024-idx gather piece

            def agg_pass(yf, rootT_sb, wroot_sb, bias_sb, post, out_cb):
                state = {"xg": None}
                for t in range(n_tiles):
                    ti = meta["tiles"][t]
                    rows, wps, k0, slot0 = ti["rows"], ti["wps"], ti["k0"], ti["slot0"]
                    C = len(wps)
                    ps = psA.tile([P, D], F32, tag="psagg")
                    # root term first: full 128 partitions (xT/hT are padded),
                    # start=True opens the accumulation group everywhere
                    nc.tensor.matmul(ps[:, :],
                                     lhsT=rootT_sb[:, t * P:(t + 1) * P],
                                     rhs=wroot_sb[:], start=True, stop=False)
                    if C > 0:
                        s_t = sp.tile([P, maxc * W], F32, tag="smat")
                        s3 = AP(s_t.tensor, s_t.offset,
                                [s_t.ap[0], [W, C], [1, W]])
                        i3 = _bcast3(iota_f[:, :], None, [[0, C], [1, W]])
                        d3 = _bcast3(dv_sb[:, k0:k0 + C], None, [[1, C], [0, W]])
                        nc.vector.tensor_tensor(out=s3, in0=i3, in1=d3,
                                                op=mybir.AluOpType.is_equal)
                        for j, wp in enumerate(wps):
                            k = k0 + j  # global chunk index
                            if k % GPC == 0:
                                state["xg"] = xgp.tile([P, GPC, D], F32,
                                                       tag="xg", name="xgbuf")
                                p0 = k * P
                                nc.gpsimd.dma_gather(
                                    state["xg"][:, :, :], yf[:, :],
                                    gidx_sb[:, p0 // 16:(p0 + GPC * P) // 16],
                                    GPC * P, GPC * P, D)
                            nc.tensor.matmul(
                                ps[wp:wp + W, :],
                                lhsT=s_t[:, j * W:(j + 1) * W],
                                rhs=state["xg"][:, k % GPC, :],
                                start=False, stop=False)
                    # bias last: full coverage closes the accumulation group
                    nc.tensor.matmul(ps[:, :], lhsT=ones_sb[:1, :],
                                     rhs=bias_sb[:1, :], start=False, stop=True)
                    o_sb = stagep.tile([P, D], F32, tag="ostage")
                    nc.scalar.activation(o_sb[:rows, :], ps[:rows, :], post)
                    out_cb(t, rows, o_sb)

            # ---- stage B: layer-1 aggregate + relu ; fused transpose + y2 ----
            def l1_out(t, rows, h_sb):
                # transpose h tile -> hT columns
                psT = psB.tile([P, P], F32, tag="psB")
                nc.tensor.transpose(out=psT[:, :], in_=h_sb[:, :],
                                    identity=ident_sb[:])
                nc.vector.tensor_copy(hT_sb[:, t * P:t * P + P], psT[:, :])
                # y2 tile = h @ W2_rel
                ps2 = psB.tile([P, D], F32, tag="psB")
                nc.tensor.matmul(ps2[:rows, :],
                                 lhsT=hT_sb[:, tslice(t, rows)],
                                 rhs=w2r_sb[:], start=True, stop=True)
                y_sb = stagep.tile([P, D], F32, tag="ystage")
                nc.scalar.activation(y_sb[:rows, :], ps2[:rows, :],
                                     mybir.ActivationFunctionType.Copy)
                nc.sync.dma_start(y2b[tslice(t, rows), :], y_sb[:rows, :])

            agg_pass(y1f, xT_sb, w1o_sb, b1_sb,
                     mybir.ActivationFunctionType.Relu, l1_out)

            nc.gpsimd.collective_compute(
                "AllGather", mybir.AluOpType.bypass, replica_groups=rg,
                ins=[y2b.opt()], outs=[y2f.opt()])

            # ---- stage D: layer-2 aggregate (no relu) -> out ----
            def l2_out(t, rows, o_sb):
                nc.sync.dma_start(out[tslice(t, rows), :], o_sb[:rows, :])

            agg_pass(y2f, hT_sb, w2o_sb, b2_sb,
                     mybir.ActivationFunctionType.Copy, l2_out)

    nc.compile()
    return nc


# ---------------------------------------------------------------------------
# Full-input wrapper
# ---------------------------------------------------------------------------

def run(inputs, n_cores=8, trace=False):
    _apply_cc_workaround()
    x = np.asarray(inputs["x"], dtype=np.float32)
    n_nodes = x.shape[0]
    meta, per_core = preprocess(inputs["edge_index"], n_nodes, n_cores)
    nc = build_kernel(meta)
    npc = meta["npc"]

    in_maps = []
    npc_pad = meta["npc_pad"]
    for c in range(n_cores):
        xs = x[c * npc:(c + 1) * npc]  # [npc, 128]
        xs_t = np.zeros((D, npc_pad), np.float32)
        xs_t[:, :npc] = xs.T
        in_maps.append({
            "xT": xs_t,
            "w1r": np.asarray(inputs["W1_rel"], np.float32),
            "w1o": np.asarray(inputs["W1_root"], np.float32),
            "w2r": np.asarray(inputs["W2_rel"], np.float32),
            "w2o": np.asarray(inputs["W2_root"], np.float32),
            "b1": np.asarray(inputs["b1_rel"], np.float32).reshape(1, D),
            "b2": np.asarray(inputs["b2_rel"], np.float32).reshape(1, D),
            "gidx": per_core[c]["gidx"],
            "dvals": per_core[c]["dval"],
        })
    res = run_bass_kernel_spmd(nc, in_maps, core_ids=list(range(n_cores)),
                               trace=trace)
    outp = np.concatenate([res.results[c]["out"] for c in range(n_cores)],
                          axis=0)
    return outp, res



def kernel(**inputs):
    out, _ = run(inputs, n_cores=8)
    return np.asarray(out, dtype=np.float32)

